# revision 1
# baseline (speedup 1.0000x reference)
"""Trainium2 Bass kernel for MHA with query-axis softmax (nn_MHA_2568390443327).

Reference computation (B=4, N=2048, DIM=1024, 16 heads x 64):
    qkv = x @ w_qkv ; q,k,v = split(qkv)
    scores = (q @ k^T) * scale            # [b,h,i(query),j(key)]
    attn = softmax(scores, axis=QUERY)    # normalized over i, per key j
    y = attn @ v ; out = y @ w_out + b_out

Sharding (8 cores): batch (4) x head-half (2). Each core gets its batch's
x (pre-transposed), the qkv weight columns and w_out rows for its 8 heads,
and produces a partial [DIM, N] output (transposed). Host sums the two
head-half partials per batch and transposes back.

Per-core math trick: scores are computed transposed, S_T[j, i] (key j on
partitions), so the query-axis softmax is a free-axis exp+sum done on the
Scalar engine with a fused accumulator (denominator), and the 1/denominator
folds into a per-row rescale of v (tiny) instead of the NxN attention
matrix. No max subtraction is needed (scores*scale ~ N(0,1); exp is safe
in fp32).

All matmul operands are float16 (full PE rate, fp32 PSUM accumulation;
~5e-4 element precision, far better than bf16).
"""

import os
import numpy as np

# ---------------------------------------------------------------------------
# Problem constants (hardcoded; kernel.py must be self-contained).
B = 4
N = 2048          # sequence length
F = 1024          # model dim (contraction for qkv proj)
HEADS_TOT = 16
DH = 64           # head dim
HH = 8            # heads per core (head-half)
CH = HH * DH      # 512: per-core hidden
OUT = 1024        # output dim
SCALE = 0.125     # 1/sqrt(64)
N_CORES = 8

P = 128           # partitions
NC512 = 512       # matmul free-dim chunk
S_W = 1024        # scores PSUM tile width (2 banks), 2 per (head, j-tile)


def _build_nc():
    import concourse.bass as bass  # noqa: F401
    import concourse.mybir as mybir
    from concourse import bacc
    from concourse.tile import TileContext

    f32 = mybir.dt.float32
    f16 = mybir.dt.float16
    EXP = mybir.ActivationFunctionType.Exp

    nc = bacc.Bacc(None, target_bir_lowering=False)

    xT = nc.declare_dram_parameter("xT", [F, N], f16, isOutput=False)
    wqkv = nc.declare_dram_parameter("wqkv", [F, 3 * CH], f16, isOutput=False)
    wout = nc.declare_dram_parameter("wout", [CH, OUT], f16, isOutput=False)
    bias = nc.declare_dram_parameter("bias", [P, OUT // P], f32, isOutput=False)
    outT = nc.declare_dram_parameter("outT", [OUT, N], f32, isOutput=True)

    KT = F // P            # 8 k-tiles for qkv projection
    NT = N // P            # 16 n(row)-tiles / j-tiles
    QKT = CH // P          # 4 c-tiles per q / k section
    PAIRS = QKT            # 4 head pairs per core
    OT = OUT // P          # 8 output row tiles
    HLV = N // S_W         # 2 halves of the i range
    C2N = S_W // NC512     # 2 chunks of 512 per half

    with TileContext(nc) as tc:
        with (
            tc.tile_pool(name="p_qkT", bufs=1) as p_qkT,
            tc.tile_pool(name="p_v", bufs=1) as p_v,
            tc.tile_pool(name="p_small", bufs=1) as p_small,
            tc.tile_pool(name="p_x", bufs=1) as p_x,
            tc.tile_pool(name="p_w", bufs=1) as p_w,
            tc.tile_pool(name="p_wout", bufs=1) as p_wout,
            tc.tile_pool(name="p_ysb", bufs=1) as p_ysb,
            tc.tile_pool(name="p_at", bufs=4) as p_at,
            tc.tile_pool(name="p_vp", bufs=4) as p_vp,
            tc.tile_pool(name="p_den", bufs=12) as p_den,
            tc.tile_pool(name="p_osb", bufs=4) as p_osb,
            tc.tile_pool(name="psMM", bufs=2, space="PSUM") as psMM,
            tc.tile_pool(name="psY", bufs=1, space="PSUM") as psY,
        ):
            bias_sb = p_small.tile([P, OUT // P], f32, tag="bias",
                                   name="bias_sb")
            nc.sync.dma_start(out=bias_sb, in_=bias[:, :])

            qT = [p_qkT.tile([P, N], f16, tag=f"qT{i}", name=f"qT{i}")
                  for i in range(QKT)]
            kT = [p_qkT.tile([P, N], f16, tag=f"kT{i}", name=f"kT{i}")
                  for i in range(QKT)]
            vnat = [p_v.tile([P, CH], f32, tag=f"v{j}", name=f"v{j}")
                    for j in range(NT)]
            xt = [p_x.tile([P, N], f16, tag=f"x{k}", name=f"x{k}")
                  for k in range(KT)]
            wt = [p_w.tile([P, 3 * CH], f16, tag=f"w{k}", name=f"w{k}")
                  for k in range(KT)]
            wout_sb = [p_wout.tile([P, OUT], f16, tag=f"wo{c}", name=f"wo{c}")
                       for c in range(QKT)]
            y_sb = [p_ysb.tile([P, N], f16, tag=f"y{p_}", name=f"y{p_}")
                    for p_ in range(PAIRS)]

            for k in range(KT):
                nc.sync.dma_start(out=xt[k], in_=xT[k * P:(k + 1) * P, :])
                nc.sync.dma_start(out=wt[k], in_=wqkv[k * P:(k + 1) * P, :])
            for c in range(QKT):
                nc.sync.dma_start(out=wout_sb[c],
                                  in_=wout[c * P:(c + 1) * P, :])

            # ---- v in natural layout: v[n, c] = sum_f xT[f, n] w_v[f, c]
            for j in range(NT):
                ps = psMM.tile([P, S_W], f32, tag="mm", name=f"psv{j}")
                for k in range(KT):
                    nc.tensor.matmul(
                        ps[:, 0:CH],
                        lhsT=xt[k][:, j * P:(j + 1) * P],
                        rhs=wt[k][:, 2 * CH:3 * CH],
                        start=(k == 0), stop=(k == KT - 1))
                nc.vector.tensor_copy(vnat[j], ps[:, 0:CH])

            # ---- q/k projection for one pair: 4 psum groups of [128, S_W]
            def emit_qk_group(pr, g):
                sec, hf = divmod(g, HLV)        # sec 0=q, 1=k
                dst = qT[pr] if sec == 0 else kT[pr]
                ps = psMM.tile([P, S_W], f32, tag="mm", name=f"qk{pr}_{g}")
                for c2 in range(C2N):
                    for k in range(KT):
                        nc.tensor.matmul(
                            ps[:, c2 * NC512:(c2 + 1) * NC512],
                            lhsT=wt[k][:, sec * CH + pr * P:
                                       sec * CH + (pr + 1) * P],
                            rhs=xt[k][:, hf * S_W + c2 * NC512:
                                      hf * S_W + (c2 + 1) * NC512],
                            start=(k == 0), stop=(k == KT - 1))
                nc.vector.tensor_copy(
                    dst[:, hf * S_W:(hf + 1) * S_W], ps)

            for g in range(2 * HLV):
                emit_qk_group(0, g)

            # ---- attention: heads A/B of each pair interleaved on the PE
            for pr in range(PAIRS):
                y_ps = psY.tile([P, N], f32, tag="Y", name=f"yps{pr}")
                for j in range(NT):
                    js = slice(j * P, (j + 1) * P)
                    ats = {0: [], 64: []}
                    dens = {0: [], 64: []}
                    for hf in range(HLV):
                        s_ps = {}
                        for ho in (0, 64):
                            s_ps[ho] = psMM.tile([P, S_W], f32, tag="mm",
                                                 name=f"s{ho}_{hf}")
                        for c2 in range(C2N):
                            i0 = hf * S_W + c2 * NC512
                            for ho in (0, 64):
                                nc.tensor.matmul(
                                    s_ps[ho][:, c2 * NC512:(c2 + 1) * NC512],
                                    lhsT=kT[pr][ho:ho + DH, js],
                                    rhs=qT[pr][ho:ho + DH, i0:i0 + NC512],
                                    start=True, stop=True,
                                    tile_position=(ho, 0))
                        for ho in (0, 64):
                            at = p_at.tile([P, S_W], f16, tag="at",
                                           name=f"at{ho}_{hf}")
                            den = p_den.tile([P, 1], f32, tag="den",
                                             name=f"den{ho}_{hf}")
                            nc.scalar.activation(at, s_ps[ho], EXP,
                                                 scale=SCALE, accum_out=den)
                            ats[ho].append(at)
                            dens[ho].append(den)
                    vp = {}
                    for ho in (0, 64):
                        dtot = p_den.tile([P, 1], f32, tag="den", name="dtot")
                        nc.vector.tensor_add(dtot, dens[ho][0], dens[ho][1])
                        rec = p_den.tile([P, 1], f32, tag="den", name="rec")
                        nc.vector.reciprocal(rec, dtot)
                        vp[ho] = p_vp.tile([P, DH], f16, tag="vp",
                                           name=f"vp{ho}")
                        c0 = pr * 2 * DH + ho
                        nc.vector.tensor_scalar_mul(
                            vp[ho], vnat[j][:, c0:c0 + DH], rec)
                    for hf in range(HLV):
                        for c2 in range(C2N):
                            i0 = hf * S_W + c2 * NC512
                            for ho in (0, 64):
                                nc.tensor.matmul(
                                    y_ps[ho:ho + DH, i0:i0 + NC512],
                                    lhsT=vp[ho],
                                    rhs=ats[ho][hf][:, c2 * NC512:
                                                    (c2 + 1) * NC512],
                                    start=(j == 0), stop=(j == NT - 1),
                                    tile_position=(0, ho))
                    # thread the next pair's q/k projection into PE slack
                    if pr + 1 < PAIRS and j % 4 == 0:
                        emit_qk_group(pr + 1, j // 4)
                nc.vector.tensor_copy(y_sb[pr], y_ps)

            # ---- output projection: outT[o, n] = sum_c wout[c, o] y[c, n]
            for o in range(OT):
                for ich in range(N // NC512):
                    ps = psMM.tile([P, NC512], f32, tag="mm",
                                   name=f"po{o}_{ich}")
                    for c in range(QKT):
                        nc.tensor.matmul(
                            ps,
                            lhsT=wout_sb[c][:, o * P:(o + 1) * P],
                            rhs=y_sb[c][:, ich * NC512:(ich + 1) * NC512],
                            start=(c == 0), stop=(c == QKT - 1))
                    osb = p_osb.tile([P, NC512], f32, tag="osb", name="osb")
                    nc.vector.tensor_scalar_add(osb, ps, bias_sb[:, o:o + 1])
                    nc.sync.dma_start(
                        out=outT[o * P:(o + 1) * P,
                                 ich * NC512:(ich + 1) * NC512],
                        in_=osb)
    return nc


def _shard_inputs(x, w_qkv, w_out, b_out):
    """Build per-core input maps: core c -> (batch c//2, head-half c%2)."""
    in_maps = []
    for c in range(N_CORES):
        b, hh = c // 2, c % 2
        cols = slice(hh * CH, (hh + 1) * CH)
        xTc = np.ascontiguousarray(np.asarray(x[b]).T, dtype=np.float16)
        wq = w_qkv[:, 0 * F:1 * F][:, cols]
        wk = w_qkv[:, 1 * F:2 * F][:, cols]
        wv = w_qkv[:, 2 * F:3 * F][:, cols]
        wqkv_c = np.ascontiguousarray(
            np.concatenate([wq, wk, wv], axis=1), dtype=np.float16)
        wout_c = np.ascontiguousarray(w_out[cols, :], dtype=np.float16)
        bias_c = np.ascontiguousarray(
            (np.asarray(b_out, dtype=np.float32) / 2.0)
            .reshape(OUT // P, P).T)
        in_maps.append(
            {"xT": xTc, "wqkv": wqkv_c, "wout": wout_c, "bias": bias_c})
    return in_maps


def _gather_outputs(results):
    out = np.empty((B, N, OUT), np.float32)
    for b in range(B):
        acc = results[2 * b]["outT"] + results[2 * b + 1]["outT"]  # [OUT, N]
        out[b] = acc.T
    return out


# Test instrumentation (harness just calls kernel(); these stay default).
_TRACE = False
_LAST_RESULT = None


def kernel(x, w_qkv, w_out, b_out):
    global _LAST_RESULT
    # The bass->PJRT path needs the axon trn2 devices visible to jax.
    if os.environ.get("JAX_PLATFORMS") not in (None, "", "axon"):
        os.environ.pop("JAX_PLATFORMS", None)
    from concourse.bass_utils import run_bass_kernel_spmd

    nc = _build_nc()
    if not nc.is_finalized():
        nc.finalize()  # runs Bacc legalization (wait splitting, reg alloc)
    in_maps = _shard_inputs(np.asarray(x), np.asarray(w_qkv),
                            np.asarray(w_out), np.asarray(b_out))
    res = run_bass_kernel_spmd(nc, in_maps, list(range(N_CORES)),
                               trace=_TRACE)
    _LAST_RESULT = res
    return _gather_outputs(res.results)


# ---------------------------------------------------------------------------
# Numpy emulation of the per-core device program (for host-logic testing;
# not used by kernel()).
def _emulate_core(m):
    xT, wqkv, wout, bias = m["xT"], m["wqkv"], m["wout"], m["bias"]
    qT = (wqkv[:, 0:CH].T @ xT)          # [CH, N]
    kTm = (wqkv[:, CH:2 * CH].T @ xT)    # [CH, N]
    v = xT.T @ wqkv[:, 2 * CH:3 * CH]    # [N, CH]
    outT_acc = np.zeros((OUT, N), np.float32)
    y = np.empty((CH, N), np.float32)
    for h in range(HH):
        qh = qT[h * DH:(h + 1) * DH, :]      # [DH, N(i)]
        kh = kTm[h * DH:(h + 1) * DH, :]     # [DH, N(j)]
        sT = kh.T @ qh                       # [j, i]
        e = np.exp(sT * SCALE)
        den = e.sum(axis=1, keepdims=True)   # over queries i, per key j
        vp = v[:, h * DH:(h + 1) * DH] / den
        y[h * DH:(h + 1) * DH, :] = vp.T @ e  # [DH, i]
    outT_acc = wout.T @ y                    # [OUT, N]
    outT_acc += bias.T.reshape(OUT, 1)
    return outT_acc


def _kernel_emulated(x, w_qkv, w_out, b_out):
    in_maps = _shard_inputs(np.asarray(x), np.asarray(w_qkv),
                            np.asarray(w_out), np.asarray(b_out))
    results = [{"outT": _emulate_core(m)} for m in in_maps]
    return _gather_outputs(results)



# revision 3
# speedup vs baseline: 1.0494x; 1.0494x over previous
"""Trainium2 Bass kernel for MHA with query-axis softmax (nn_MHA_2568390443327).

Reference computation (B=4, N=2048, DIM=1024, 16 heads x 64):
    qkv = x @ w_qkv ; q,k,v = split(qkv)
    scores = (q @ k^T) * scale            # [b,h,i(query),j(key)]
    attn = softmax(scores, axis=QUERY)    # normalized over i, per key j
    y = attn @ v ; out = y @ w_out + b_out

Sharding (8 cores): batch (4) x head-half (2). Each core gets its batch's
x (pre-transposed), the qkv weight columns and w_out rows for its 8 heads,
and produces a partial [DIM, N] output (transposed, f16). Host sums the two
head-half partials per batch in f32 and transposes back.

Per-core schedule (the perf-critical part):
  - Scores are computed transposed S_T[j, i] so the query-axis softmax is a
    free-axis exp+row-sum on the Scalar engine (fused accumulator), and the
    1/denominator folds into a tiny per-row rescale of v.
  - The two heads of a pair run CONCURRENTLY on the PE via tile_position
    row-packing (scores, K=64) and col-packing (attn@v, M=64) -- measured
    ~1.75x on this hardware when the instructions' waits are pre-satisfied.
  - The main loop is software-pipelined over 64 (pair, j) steps with
    attn@v lagged 2 steps behind scores, so the PE never waits on the
    Scalar engine's exp chain; filler matmuls (v-proj, next pair's q/k
    proj, previous pair's out-proj partial) keep the PE dense so the HAM
    clock gate stays at 2.4 GHz.
  - Out-proj partials for pairs 0..2 are accumulated into SBUF (f16)
    during later pairs' attention, so the serial tail is only pair 3's
    32 matmuls + bias + DMA out.
"""

import os
import numpy as np

# ---------------------------------------------------------------------------
# Problem constants (hardcoded; kernel.py must be self-contained).
B = 4
N = 2048          # sequence length
F = 1024          # model dim (contraction for qkv proj)
HEADS_TOT = 16
DH = 64           # head dim
HH = 8            # heads per core (head-half)
CH = HH * DH      # 512: per-core hidden
OUT = 1024        # output dim
SCALE = 0.125     # 1/sqrt(64)
N_CORES = 8

P = 128           # partitions
NC512 = 512       # matmul free-dim chunk
S_W = 1024        # scores PSUM tile width (2 banks)
PAIRS = 4         # head pairs per core
NT = N // P       # 16 j-tiles
KT = F // P       # 8 k-tiles for qkv projection
OT = OUT // P     # 8 output row tiles
LAG = 2           # attn@v runs LAG steps behind scores


def _build_nc():
    import concourse.bass as bass  # noqa: F401
    import concourse.mybir as mybir
    from concourse import bacc
    from concourse.tile import TileContext

    f32 = mybir.dt.float32
    f16 = mybir.dt.float16
    EXP = mybir.ActivationFunctionType.Exp
    ADD = mybir.AluOpType.add

    nc = bacc.Bacc(None, target_bir_lowering=False)

    xT = nc.declare_dram_parameter("xT", [F, N], f16, isOutput=False)
    wqkv = nc.declare_dram_parameter("wqkv", [F, 3 * CH], f16, isOutput=False)
    wout = nc.declare_dram_parameter("wout", [CH, OUT], f16, isOutput=False)
    bias = nc.declare_dram_parameter("bias", [P, OUT // P], f32,
                                     isOutput=False)
    outT = nc.declare_dram_parameter("outT", [OUT, N], f16, isOutput=True)

    with TileContext(nc) as tc:
        with (
            tc.tile_pool(name="p_x", bufs=1) as p_x,
            tc.tile_pool(name="p_w", bufs=1) as p_w,
            tc.tile_pool(name="p_qkT", bufs=1) as p_qkT,
            tc.tile_pool(name="p_v", bufs=1) as p_v,
            tc.tile_pool(name="p_wout", bufs=1) as p_wout,
            tc.tile_pool(name="p_small", bufs=1) as p_small,
            tc.tile_pool(name="p_ysb", bufs=1) as p_ysb,
            tc.tile_pool(name="p_oacc", bufs=1) as p_oacc,
            tc.tile_pool(name="p_at", bufs=12) as p_at,
            tc.tile_pool(name="p_vp", bufs=8) as p_vp,
            tc.tile_pool(name="p_den", bufs=24) as p_den,
            tc.tile_pool(name="p_osb", bufs=4) as p_osb,
            tc.tile_pool(name="psMM", bufs=2, space="PSUM") as psMM,
            tc.tile_pool(name="psY", bufs=1, space="PSUM") as psY,
        ):
            xt = [p_x.tile([P, N], f16, tag=f"x{k}", name=f"x{k}")
                  for k in range(KT)]
            wt = [p_w.tile([P, 3 * CH], f16, tag=f"w{k}", name=f"w{k}")
                  for k in range(KT)]
            qT = [p_qkT.tile([P, N], f16, tag=f"qT{i}", name=f"qT{i}")
                  for i in range(PAIRS)]
            kT = [p_qkT.tile([P, N], f16, tag=f"kT{i}", name=f"kT{i}")
                  for i in range(PAIRS)]
            vnat = [p_v.tile([P, CH], f16, tag=f"v{j}", name=f"v{j}")
                    for j in range(NT)]
            wout_sb = [p_wout.tile([P, OUT], f16, tag=f"wo{c}",
                                   name=f"wo{c}") for c in range(PAIRS)]
            y_sb = [p_ysb.tile([P, N], f16, tag=f"y{p_}", name=f"y{p_}")
                    for p_ in range(PAIRS)]
            oacc = [p_oacc.tile([P, N], f16, tag=f"oa{o}", name=f"oa{o}")
                    for o in range(OT)]
            bias_sb = p_small.tile([P, OUT // P], f32, tag="bias",
                                   name="bias_sb")

            # ---- input DMA: x/w k-tiles interleaved so the qk projection
            # matmuls pipeline right behind the transfers
            for k in range(KT):
                nc.sync.dma_start(out=xt[k], in_=xT[k * P:(k + 1) * P, :])
                nc.sync.dma_start(out=wt[k], in_=wqkv[k * P:(k + 1) * P, :])
            for c in range(PAIRS):
                nc.sync.dma_start(out=wout_sb[c],
                                  in_=wout[c * P:(c + 1) * P, :])
            nc.sync.dma_start(out=bias_sb, in_=bias[:, :])

            # ---- q/k projection for (pair, sec, hf): [128, 1024] psum
            def emit_qk_group(pr, sec, hf):
                dst = qT[pr] if sec == 0 else kT[pr]
                ps = psMM.tile([P, S_W], f32, tag="mm", name=f"qk{pr}{sec}{hf}")
                for c2 in range(2):
                    for k in range(KT):
                        nc.tensor.matmul(
                            ps[:, c2 * NC512:(c2 + 1) * NC512],
                            lhsT=wt[k][:, sec * CH + pr * P:
                                       sec * CH + (pr + 1) * P],
                            rhs=xt[k][:, hf * S_W + c2 * NC512:
                                      hf * S_W + (c2 + 1) * NC512],
                            start=(k == 0), stop=(k == KT - 1))
                nc.vector.tensor_copy(dst[:, hf * S_W:(hf + 1) * S_W], ps)

            # two-group-interleaved variant used in the head so the k-loop
            # pipelines behind the x/w DMAs
            def emit_qk_groups_interleaved(specs):
                tiles = {}
                for gi, (pr, sec, hf) in enumerate(specs):
                    tiles[gi] = psMM.tile([P, S_W], f32, tag="mm",
                                          name=f"qkh{pr}{sec}{hf}")
                for k in range(KT):
                    for gi, (pr, sec, hf) in enumerate(specs):
                        for c2 in range(2):
                            nc.tensor.matmul(
                                tiles[gi][:, c2 * NC512:(c2 + 1) * NC512],
                                lhsT=wt[k][:, sec * CH + pr * P:
                                           sec * CH + (pr + 1) * P],
                                rhs=xt[k][:, hf * S_W + c2 * NC512:
                                          hf * S_W + (c2 + 1) * NC512],
                                start=(k == 0), stop=(k == KT - 1))
                for gi, (pr, sec, hf) in enumerate(specs):
                    dst = qT[pr] if sec == 0 else kT[pr]
                    nc.vector.tensor_copy(dst[:, hf * S_W:(hf + 1) * S_W],
                                          tiles[gi])

            # ---- v projection for one j-tile: [128, 512] psum
            def emit_v(j):
                ps = psMM.tile([P, NC512], f32, tag="mm", name=f"v{j}")
                for k in range(KT):
                    nc.tensor.matmul(
                        ps,
                        lhsT=xt[k][:, j * P:(j + 1) * P],
                        rhs=wt[k][:, 2 * CH:2 * CH + NC512],
                        start=(k == 0), stop=(k == KT - 1))
                nc.vector.tensor_copy(vnat[j], ps)

            # ---- scores + exp for step s; returns per-step state
            def emit_scores(s):
                pr, j = divmod(s, NT)
                js = slice(j * P, (j + 1) * P)
                ats = {0: [], 64: []}
                dens = {0: [], 64: []}
                for hf in range(2):
                    tA = psMM.tile([P, S_W], f32, tag="mm", name=f"sA{s}{hf}")
                    tB = psMM.tile([P, S_W], f32, tag="mm", name=f"sB{s}{hf}")
                    for c2 in range(2):
                        cs = slice(c2 * NC512, (c2 + 1) * NC512)
                        i0 = hf * S_W + c2 * NC512
                        nc.tensor.matmul(
                            tA[:, cs], lhsT=kT[pr][0:DH, js],
                            rhs=qT[pr][0:DH, i0:i0 + NC512],
                            start=True, stop=True, tile_position=(0, 0))
                        nc.tensor.matmul(
                            tB[:, cs], lhsT=kT[pr][DH:P, js],
                            rhs=qT[pr][DH:P, i0:i0 + NC512],
                            start=True, stop=True, tile_position=(64, 0))
                    for ho, t in ((0, tA), (64, tB)):
                        at = p_at.tile([P, S_W], f16, tag="at",
                                       name=f"at{s}{hf}{ho}")
                        den = p_den.tile([P, 1], f32, tag="den",
                                         name=f"dn{s}{hf}{ho}")
                        nc.scalar.activation(at, t, EXP, scale=SCALE,
                                             accum_out=den)
                        ats[ho].append(at)
                        dens[ho].append(den)
                return {"pr": pr, "j": j, "ats": ats, "dens": dens}

            # ---- denominator finalize + v rescale for step s (DVE)
            def emit_vp(st):
                j = st["j"]
                pr = st["pr"]
                st["vp"] = {}
                for ho in (0, 64):
                    dtot = p_den.tile([P, 1], f32, tag="den", name="dtot")
                    nc.vector.tensor_add(dtot, st["dens"][ho][0],
                                         st["dens"][ho][1])
                    rec = p_den.tile([P, 1], f32, tag="den", name="rec")
                    nc.vector.reciprocal(rec, dtot)
                    vp = p_vp.tile([P, DH], f16, tag="vp", name=f"vp{ho}")
                    c0 = pr * 2 * DH + ho
                    nc.vector.tensor_scalar_mul(
                        vp, vnat[j][:, c0:c0 + DH], rec)
                    st["vp"][ho] = vp

            # ---- attn@v for a completed step (col-packed pairs)
            def emit_attnv(st, y_ps):
                j = st["j"]
                for hf in range(2):
                    for c2 in range(2):
                        i0 = hf * S_W + c2 * NC512
                        cs = slice(c2 * NC512, (c2 + 1) * NC512)
                        for ho in (0, 64):
                            nc.tensor.matmul(
                                y_ps[ho:ho + DH, i0:i0 + NC512],
                                lhsT=st["vp"][ho],
                                rhs=st["ats"][ho][hf][:, cs],
                                start=(j == 0), stop=(j == NT - 1),
                                tile_position=(0, ho))

            # ---- out-projection partial for pair pr into oacc (f16 SBUF)
            # chunk = list of (o, ich)
            def emit_outproj_partial(pr, chunk):
                for (o, ich) in chunk:
                    ps = psMM.tile([P, NC512], f32, tag="mm",
                                   name=f"po{pr}{o}{ich}")
                    ics = slice(ich * NC512, (ich + 1) * NC512)
                    nc.tensor.matmul(
                        ps,
                        lhsT=wout_sb[pr][:, o * P:(o + 1) * P],
                        rhs=y_sb[pr][:, ics],
                        start=True, stop=True)
                    if pr == 0:
                        nc.vector.tensor_copy(oacc[o][:, ics], ps)
                    else:
                        nc.vector.tensor_add(oacc[o][:, ics], ps,
                                             oacc[o][:, ics])

            # ---- final out-projection (pair 3) + bias + oacc + DMA out
            def emit_outproj_final(chunk):
                for (o, ich) in chunk:
                    ps = psMM.tile([P, NC512], f32, tag="mm",
                                   name=f"pf{o}{ich}")
                    ics = slice(ich * NC512, (ich + 1) * NC512)
                    nc.tensor.matmul(
                        ps,
                        lhsT=wout_sb[3][:, o * P:(o + 1) * P],
                        rhs=y_sb[3][:, ics],
                        start=True, stop=True)
                    osb = p_osb.tile([P, NC512], f16, tag="osb", name="osb")
                    nc.vector.scalar_tensor_tensor(
                        osb, ps, bias_sb[:, o:o + 1], oacc[o][:, ics],
                        op0=ADD, op1=ADD)
                    nc.sync.dma_start(
                        out=outT[o * P:(o + 1) * P, ics], in_=osb)

            # =========== head: qk projection for pair 0 + first v tiles
            emit_qk_groups_interleaved([(0, 0, 0), (0, 1, 0)])
            emit_qk_groups_interleaved([(0, 0, 1), (0, 1, 1)])
            for j in range(LAG):
                emit_v(j)

            # filler schedule: per pair, list of (step_offset, fn) emitted
            # after that step's attnv
            # pair 0: v-proj j=2..15 (14 tiles) + qk(1) 4 groups
            # pair 1: qk(2) + outproj(0);  pair 2: qk(3) + outproj(1)
            # pair 3: outproj(2)
            fillers = {s: [] for s in range(64)}
            for jj in range(LAG, NT):          # v2..v15 at steps 0..13
                fillers[jj - LAG].append(lambda j=jj: emit_v(j))
            for g, (sec, hf) in enumerate(
                    [(0, 0), (1, 0), (0, 1), (1, 1)]):
                fillers[2 * g + 7].append(
                    lambda s_=sec, h_=hf: emit_qk_group(1, s_, h_))
            for pr in (1, 2):
                for g, (sec, hf) in enumerate(
                        [(0, 0), (1, 0), (0, 1), (1, 1)]):
                    fillers[pr * NT + 2 * g + 1].append(
                        lambda s_=sec, h_=hf, p_=pr + 1:
                        emit_qk_group(p_, s_, h_))
            oplist = [(o, ich) for o in range(OT) for ich in range(4)]
            for pr in (1, 2, 3):
                for ci in range(4):            # 4 chunks of 8
                    chunk = oplist[ci * 8:(ci + 1) * 8]
                    fillers[pr * NT + 2 * ci + 8].append(
                        lambda c_=chunk, p_=pr - 1:
                        emit_outproj_partial(p_, c_))

            # =========== main software-pipelined loop
            states = {}
            y_ps = None
            for s in range(64 + LAG):
                if s < 64:
                    states[s] = emit_scores(s)
                t = s - LAG
                if t >= 0:
                    st = states.pop(t)
                    if st["j"] == 0:
                        y_ps = psY.tile([P, N], f32, tag="Y",
                                        name=f"yps{st['pr']}")
                    emit_attnv(st, y_ps)
                    if st["j"] == NT - 1:
                        nc.vector.tensor_copy(y_sb[st["pr"]], y_ps)
                if s < 64:
                    emit_vp(states[s])
                    for fn in fillers[s]:
                        fn()

            # =========== tail: pair-3 out-projection
            for ci in range(4):
                emit_outproj_final(oplist[ci * 8:(ci + 1) * 8])
    return nc


def _shard_inputs(x, w_qkv, w_out, b_out):
    """Build per-core input maps: core c -> (batch c//2, head-half c%2)."""
    in_maps = []
    for c in range(N_CORES):
        b, hh = c // 2, c % 2
        cols = slice(hh * CH, (hh + 1) * CH)
        xTc = np.ascontiguousarray(np.asarray(x[b]).T, dtype=np.float16)
        wq = w_qkv[:, 0 * F:1 * F][:, cols]
        wk = w_qkv[:, 1 * F:2 * F][:, cols]
        wv = w_qkv[:, 2 * F:3 * F][:, cols]
        wqkv_c = np.ascontiguousarray(
            np.concatenate([wq, wk, wv], axis=1), dtype=np.float16)
        wout_c = np.ascontiguousarray(w_out[cols, :], dtype=np.float16)
        bias_c = np.ascontiguousarray(
            (np.asarray(b_out, dtype=np.float32) / 2.0)
            .reshape(OUT // P, P).T)
        in_maps.append(
            {"xT": xTc, "wqkv": wqkv_c, "wout": wout_c, "bias": bias_c})
    return in_maps


def _gather_outputs(results):
    out = np.empty((B, N, OUT), np.float32)
    for b in range(B):
        acc = (results[2 * b]["outT"].astype(np.float32)
               + results[2 * b + 1]["outT"].astype(np.float32))  # [OUT, N]
        out[b] = acc.T
    return out


# Test instrumentation (harness just calls kernel(); these stay default).
_TRACE = False
_LAST_RESULT = None


def kernel(x, w_qkv, w_out, b_out):
    global _LAST_RESULT
    # The bass->PJRT path needs the axon trn2 devices visible to jax.
    if os.environ.get("JAX_PLATFORMS") not in (None, "", "axon"):
        os.environ.pop("JAX_PLATFORMS", None)
    from concourse.bass_utils import run_bass_kernel_spmd

    nc = _build_nc()
    if not nc.is_finalized():
        nc.finalize()  # runs Bacc legalization (wait splitting, reg alloc)
    in_maps = _shard_inputs(np.asarray(x), np.asarray(w_qkv),
                            np.asarray(w_out), np.asarray(b_out))
    res = run_bass_kernel_spmd(nc, in_maps, list(range(N_CORES)),
                               trace=_TRACE)
    _LAST_RESULT = res
    return _gather_outputs(res.results)


# ---------------------------------------------------------------------------
# Numpy emulation of the per-core device program (for host-logic testing;
# not used by kernel()).
def _emulate_core(m):
    xT = m["xT"].astype(np.float32)
    wqkv, wout, bias = m["wqkv"], m["wout"], m["bias"]
    qT = (wqkv[:, 0:CH].T.astype(np.float32) @ xT)          # [CH, N]
    kTm = (wqkv[:, CH:2 * CH].T.astype(np.float32) @ xT)    # [CH, N]
    v = xT.T @ wqkv[:, 2 * CH:3 * CH].astype(np.float32)    # [N, CH]
    y = np.empty((CH, N), np.float32)
    for h in range(HH):
        qh = qT[h * DH:(h + 1) * DH, :]      # [DH, N(i)]
        kh = kTm[h * DH:(h + 1) * DH, :]     # [DH, N(j)]
        sT = kh.T @ qh                       # [j, i]
        e = np.exp(sT * SCALE)
        den = e.sum(axis=1, keepdims=True)   # over queries i, per key j
        vp = v[:, h * DH:(h + 1) * DH] / den
        y[h * DH:(h + 1) * DH, :] = vp.T @ e  # [DH, i]
    outT_acc = wout.T.astype(np.float32) @ y  # [OUT, N]
    outT_acc += bias.T.reshape(OUT, 1)
    return outT_acc.astype(np.float16)


def _kernel_emulated(x, w_qkv, w_out, b_out):
    in_maps = _shard_inputs(np.asarray(x), np.asarray(w_qkv),
                            np.asarray(w_out), np.asarray(b_out))
    results = [{"outT": _emulate_core(m)} for m in in_maps]
    return _gather_outputs(results)
